# revision 3
# baseline (speedup 1.0000x reference)
"""Space-to-depth (8x8 chessboard) kernel for Trainium2.

Full input  : (32, 256, 256, 32) f32
Full output : (32, 8, 8, 32768) f32
out[b, i, j] = inputs[b, i*32:(i+1)*32, j*32:(j+1)*32, :].reshape(-1)

Sharding: batch dim (32) split across 8 NeuronCores (pure data parallel,
no communication) -> 4 examples per core.

Per core the op is pure HBM->HBM data movement, done entirely with DMA
access patterns (no compute engines). Within one (example b, 32-row
band i), iterating (r, j, elem) makes the source AP contiguous and the
destination a 3D AP, so a single DMA moves a block of rows in 4 KiB
contiguous chunks:

  src [[8192, nr], [1024, k], [1, 1024]]   (contiguous 32 KiB per row r)
  dst [[1024, nr], [32768, k], [1, 1024]]  (4 KiB chunks, 32 KiB stride)

Trace facts (NTFF, all 8 cores running):
- 16 SDMA engines per core; engine = outer AP index mod 16, so per-job
  coverage is always an engine PREFIX 0..nr-1 (loads must be
  non-increasing in engine index).
- Two HWDGE descgen rings (SP/sync + ACT/scalar) generate ~1 descriptor
  per ~25ns each; a DMA's doorbell fires when its whole descgen is done,
  so the first 128-desc job delays first data by ~3us -> ramp in with
  small jobs first.
- Steady state ~320 GB/s payload/core (~20.6 GB/s/engine HBM->HBM, the
  device HBM wall with all 8 cores active); packet durations drop to
  ~155ns (26 GB/s) when contention eases near the tail.
- Engine 15 is intermittently ~1.25x slower (trn2 quirk), so it gets a
  reduced row count chosen to finish just-in-time even when slow.
"""

import numpy as np

_B_PER_CORE = 4
_N_CORES = 8
_IN_SHAPE = (_B_PER_CORE, 256, 256, 32)
_OUT_SHAPE = (_B_PER_CORE, 8, 8, 32768)
_BAND = 32 * 256 * 32     # elements per (example, row-band)  (262144)

# rows per engine: engines 0-14 flat, engine 15 hedged (slow-engine risk)
_ROWS15 = 49              # rows for engine 15; engines 0-14 get (1024-_ROWS15)/15

_CACHE = {}


def build_nc():
    import concourse.bass as bass
    import concourse.mybir as mybir

    # Bass.__init__ ends with an all_engine_barrier that makes the DMA
    # sequencers (SP/ACT) wait for GpSimd's slow boot before the first
    # dma_start, costing ~1-3 us of ramp. Nothing in this kernel reads
    # the init-preamble state (const SBUF tensors / gpsimd), so suppress
    # that one barrier. The patch is scoped to construction and restored
    # before returning; the Block exit barrier is emitted normally.
    orig_barrier = bass.Bass.all_engine_barrier
    bass.Bass.all_engine_barrier = lambda self, **kw: None
    try:
        nc = bass.Bass(target_bir_lowering=False)
    finally:
        bass.Bass.all_engine_barrier = orig_barrier
    x = nc.dram_tensor("x", list(_IN_SHAPE), mybir.dt.float32, kind="ExternalInput")
    y = nc.dram_tensor("y", list(_OUT_SHAPE), mybir.dt.float32, kind="ExternalOutput")

    # ---- job list ----------------------------------------------------
    # Work unit: one row (band g, row r) = 32 KiB = 8 packets. 32 bands
    # x 32 rows = 1024 rows. Job types (engine = outer index):
    #   ("rows", g, r0, nr)        rows r0..r0+nr-1 of band g -> eng 0..nr-1
    #   ("cols", g, r0, j0, k)     chunks j0..j0+k-1 of rows r0..r0+15 of
    #                              band g -> eng 0..15, k packets each
    #   ("orph", g0, cnt)          row 31 of bands g0..g0+cnt-1 -> eng 0..cnt-1
    #
    # Per-engine row totals: eng0-14: 65, eng15: 49  (1024 total).
    #   49 x nr16 jobs + 16 x nr15 jobs.
    # Layout: every band: rows 0-15 full (32 jobs, nr16).
    #   17 bands (15..31): rows 16-31 full (nr16)  -> 49 nr16 total
    #   15 bands (0..14):  rows 16-30 (nr15); row 31 leftover
    #   1 orphan: row 31 of bands 0-14 (nr15)      -> 16 nr15 total
    # Ramp-in: the first rows0-15 block of each queue is split into
    # column-chunk jobs (16/32/80 descs) so the first doorbell lands
    # ~0.4us after descgen start instead of ~3.2us.
    sp_jobs = [("cols", 0, 0, 0, 1), ("cols", 0, 0, 1, 2), ("cols", 0, 0, 3, 5)]
    act_jobs = [("cols", 1, 0, 0, 1), ("cols", 1, 0, 1, 2), ("cols", 1, 0, 3, 5)]

    rest = []
    for g in range(2, 32):
        rest.append(("rows", g, 0, 16))
    for g in range(15, 32):
        rest.append(("rows", g, 16, 16))
    for g in range(0, 15):
        rest.append(("rows", g, 16, 15))
    rest.append(("orph", 0, 15))
    # interleave by descriptor count to keep the two rings balanced
    # (ramp blocks above already gave each queue 128 descs)
    sp_d = act_d = 128
    for job in rest:
        nd = 8 * job[3] if job[0] == "rows" else 8 * job[2]
        if sp_d <= act_d:
            sp_jobs.append(job)
            sp_d += nd
        else:
            act_jobs.append(job)
            act_d += nd
    # second block of each queue: rows 0-15 of bands 0/1 (covered by ramp
    # cols jobs only partially: cols jobs covered all 8 chunks? 1+2+5=8 yes)

    def issue(engine, my_jobs, sem):
        n = 0
        for job in my_jobs:
            if job[0] == "rows":
                _, g, r0, nr = job
                off = g * _BAND
                src = bass.AP(
                    x, off + r0 * 8192, [[8192, nr], [1024, 8], [1, 1024]]
                )
                dst = bass.AP(
                    y, off + r0 * 1024, [[1024, nr], [32768, 8], [1, 1024]]
                )
            elif job[0] == "cols":
                _, g, r0, j0, k = job
                off = g * _BAND
                src = bass.AP(
                    x,
                    off + r0 * 8192 + j0 * 1024,
                    [[8192, 16], [1024, k], [1, 1024]],
                )
                dst = bass.AP(
                    y,
                    off + r0 * 1024 + j0 * 32768,
                    [[1024, 16], [32768, k], [1, 1024]],
                )
            else:  # orph: row 31 of cnt consecutive bands
                _, g0, cnt = job
                src = bass.AP(
                    x, g0 * _BAND + 31 * 8192, [[_BAND, cnt], [1024, 8], [1, 1024]]
                )
                dst = bass.AP(
                    y, g0 * _BAND + 31 * 1024, [[_BAND, cnt], [32768, 8], [1, 1024]]
                )
            engine.dma_start(out=dst, in_=src).then_inc(sem, 16)
            n += 16
        if n:
            engine.wait_ge(sem, n)

    with (
        nc.semaphore("sp_sem") as sp_sem,
        nc.semaphore("act_sem") as act_sem,
        nc.Block(no_gpsimd_drain=True) as block,
    ):

        @block.sync
        def _(sync):
            issue(sync, sp_jobs, sp_sem)

        @block.scalar
        def _(scalar):
            issue(scalar, act_jobs, act_sem)

    return nc


def _get_nc():
    if "nc" not in _CACHE:
        _CACHE["nc"] = build_nc()
    return _CACHE["nc"]


def kernel(inputs: np.ndarray) -> np.ndarray:
    from concourse.bass_utils import run_bass_kernel_spmd

    inputs = np.ascontiguousarray(np.asarray(inputs, dtype=np.float32))
    assert inputs.shape == (_B_PER_CORE * _N_CORES,) + _IN_SHAPE[1:]

    nc = _get_nc()
    in_maps = [
        {"x": np.ascontiguousarray(inputs[c * _B_PER_CORE : (c + 1) * _B_PER_CORE])}
        for c in range(_N_CORES)
    ]
    res = run_bass_kernel_spmd(nc, in_maps, core_ids=list(range(_N_CORES)))
    return np.concatenate([r["y"] for r in res.results], axis=0)
